# revision 16
# baseline (speedup 1.0000x reference)
"""Trainium2 Bass kernel for the 2-block masked-attention GNN (nn_FEATURE_rec_16930761081280).

Strategy (v3)
-------------
Data-parallel over batch B=8 across 8 NeuronCores (1 graph per core).
Per core, the whole network runs out of SBUF in a transposed layout:

  - All activations are feature-major ("xT" = [128 feat, 2048 node]); every
    linear is a stationary-weight matmul chain.
  - Attention scores are computed TRANSPOSED (sT[m, i] = sum_d kT[d,m] qT[d,i])
    so softmax renormalization is deferred: the e@v contraction over m runs
    with eT chunks stationary against v_aug = [v | 1], yielding f1_unnorm and
    the row-sum in one PSUM region; normalization is a per-partition scalar
    multiply in the natural domain fused into the PSUM->SBUF move.
  - softmax uses a fixed shift C=64 (scores are relu'd-q . relu'd-k >= 0,
    bounded ~92 for this input distribution); masked entries become exact
    zeros via the multiplicative adjacency mask after exp.
  - Wo of each attention block is FOLDED into the next stage's linears on the
    host (W~q2 = Wq2 @ Wo1, b~q2 = Wq2 @ bo1 + bq2, W~fA = WfA @ Wo2,
    b~f = WfA @ bo2 + bf): the normalized+transposed attention output feeds
    the next block's q/k/v matmuls directly - no wo matmul, no wo bias.
  - Engine balance: ACT does ONLY exp (the sole exp engine at 1 col/cycle
    from fp32 PSUM - the hard floor of this kernel, ~72us).  All bias+relu,
    normalization scales and copies run on DVE; a tunable slice of the
    adjacency-mask multiplies runs on the otherwise-idle GPSIMD.  Mask
    multiplies work on merged [128,2048] double-pair tiles to halve DVE
    instruction overhead.
  - PSUM: "score" 2x[128,1024]f32, "f1t" 2x[128,258]f32 (two 129-wide
    accumulation regions per bank; matmul start=True clears the whole bank,
    so only region 0 uses start=True and region 1 relies on per-element
    has_written), "lin" 2 banks for linears/transposes.
  - ~24 warmup matmuls run during the input DMA so the PE HAM clock-gate is
    at 2.4 GHz when real work starts; block-2 q/k/v chunks and final-linear
    chunks are emitted eagerly per attention i-group (per-block tile tags
    keep the pipelines WAR-free) so the PE never idles >3.4us.

Precision: fp16 for q/k/s and linear weights (fp32 accumulate), bf16 for e/v
(exp needs the 8-bit exponent range), fp32 for biases/psum/normalization.
Measured end-to-end max-abs rel error vs the fp32 reference: ~4.8e-3.
"""

import sys

sys.path.insert(0, "/opt/trn_rl_repo")

import numpy as np
import ml_dtypes

import concourse.bass as bass
import concourse.bacc as bacc
import concourse.tile as tile
from concourse import mybir
from concourse.bass_utils import run_bass_kernel_spmd

B, N, D = 8, 2048, 128
NCORES = 8
C_SUB = 64.0  # fixed softmax shift
NM = N // 128  # 16 m-chunks
NIG = 4        # i-groups of 512
NPAIR = NM // 2
NQUAD = NPAIR // 2  # double-pairs per ig for the mask multiply

# which double-pair mask multiplies go to GPSIMD: every GP_MASK_MOD-th
GP_MASK_MOD = 10**9
N_WARM_MM = 8

f32 = mybir.dt.float32
f16 = mybir.dt.float16
bf16 = mybir.dt.bfloat16

np_bf16 = ml_dtypes.bfloat16

W_NAMES = ["wq1", "wk1", "wv1", "wq2", "wk2", "wv2", "wfA", "wfB", "ident"]
B_NAMES = ["bq1", "bk1", "bv1", "bq2", "bk2", "bv2", "bf"]


def build_nc():
    nc = bacc.Bacc(None)
    AF = mybir.ActivationFunctionType
    OP = mybir.AluOpType

    hT_d = nc.dram_tensor("hT", [D, N], f16, kind="ExternalInput")
    # adjacency double-pair tiles: [ig*NQUAD + q, 128, 2048]
    adjP_d = nc.dram_tensor("adjP", [NIG * NPAIR, 128, 1024], bf16, kind="ExternalInput")
    vaeT_d = nc.dram_tensor("vaeT", [D, N], f16, kind="ExternalInput")
    wpack_d = nc.dram_tensor("wpack", [len(W_NAMES), 128, 128], f16, kind="ExternalInput")
    bpack_d = nc.dram_tensor("bpack", [128, len(B_NAMES)], f32, kind="ExternalInput")
    outT_d = nc.dram_tensor("outT", [8, 128, 256], f16, kind="ExternalOutput")

    with tile.TileContext(nc) as tc:
        with (
            tc.tile_pool(name="const", bufs=1) as const,
            tc.tile_pool(name="adj", bufs=1) as adjp,
            tc.tile_pool(name="act", bufs=1) as actp,
            tc.tile_pool(name="small", bufs=8) as small,
            tc.tile_pool(name="e", bufs=4) as epool,
            tc.tile_pool(name="score", bufs=2, space="PSUM") as score,
            tc.tile_pool(name="f1tp", bufs=2, space="PSUM") as f1tp,
            tc.tile_pool(name="lin", bufs=2, space="PSUM") as linp,
        ):
            # ---- constants into SBUF via the sync HWDGE queues ----
            wpack = const.tile([128, len(W_NAMES) * 128], f16, tag="wpack")
            for j in range(len(W_NAMES)):
                nc.sync.dma_start(wpack[:, j * 128 : (j + 1) * 128], wpack_d[j])
            hT = const.tile([D, N], f16, tag="hT")
            for c in range(4):
                nc.sync.dma_start(hT[:, c * 512 : (c + 1) * 512],
                                  hT_d[:, c * 512 : (c + 1) * 512])
            bpack = const.tile([128, len(B_NAMES)], f32, tag="bpack")
            nc.sync.dma_start(bpack[:], bpack_d[:])

            W = {
                name: wpack[:, j * 128 : (j + 1) * 128]
                for j, name in enumerate(W_NAMES)
            }
            Bv = {name: bpack[:, j : j + 1] for j, name in enumerate(B_NAMES)}
            ident = W["ident"]

            # adjacency mask tiles (double-pair), in consumption order
            adj_t = {}
            for ig in range(NIG):
                for p in range(NPAIR):
                    t = adjp.tile([128, 1024], bf16, tag=f"adj_{ig}_{p}")
                    nc.sync.dma_start(t[:], adjP_d[ig * NPAIR + p])
                    adj_t[(ig, p)] = t

            vaeT = const.tile([D, N], f16, tag="vaeT")
            nc.sync.dma_start(vaeT[:], vaeT_d[:])

            negC = const.tile([128, 1], f32, tag="negC")
            nc.gpsimd.memset(negC[:], -C_SUB)
            # warm the ACT exp table while DMAs stream (table load ~2.7us)
            actwarm = const.tile([128, 1], f32, tag="actwarm")
            nc.scalar.activation(actwarm[:], negC[:], AF.Exp)

            # ---- PE warmup for the HAM clock-gate ----
            for wi in range(N_WARM_MM):
                wps = score.tile([128, 512], f32, tag="ps", name=f"warm_{wi}")
                nc.tensor.matmul(wps[:], ident, wpack[:, 0:512], start=True, stop=True)

            # v_aug tiles, per block: [v_m | 1 | pad | v_m+1 | 1 | pad]
            v_augs = {}
            for blk in (1, 2):
                va = []
                for p in range(NPAIR):
                    t = actp.tile([128, 260], bf16, tag=f"v_augP{blk}_{p}",
                                  name=f"v_augP{blk}_{p}")
                    nc.gpsimd.memset(t[:, 128:129], 1.0)
                    nc.gpsimd.memset(t[:, 258:259], 1.0)
                    va.append(t)
                v_augs[blk] = va

            mask_counter = [0]

            def qkv_chunk(blk, c, xT_c, qkv):
                """Emit q/k/v linears for chunk c of block blk plus the v
                transposes for pairs 2c, 2c+1."""
                sfx = str(blk)
                qTs, kTs, vTs = qkv
                for w_name, b_name, dst, on_act in (
                    ("wq" + sfx, "bq" + sfx, qTs[c], True),
                    ("wk" + sfx, "bk" + sfx, kTs[c], blk == 1),
                    ("wv" + sfx, "bv" + sfx, vTs[c], False),
                ):
                    ps = linp.tile([128, 512], f32, tag="lin", name=f"ps_{w_name}_{c}")
                    nc.tensor.matmul(ps[:], W[w_name], xT_c, start=True, stop=True)
                    if on_act:
                        nc.scalar.activation(dst[:], ps[:], AF.Relu, bias=Bv[b_name])
                    else:
                        nc.vector.tensor_scalar(
                            dst[:], ps[:], Bv[b_name], 0.0, OP.add, OP.max
                        )
                v_augP = v_augs[blk]
                for p in (2 * c, 2 * c + 1):
                    pt = linp.tile([128, 256], f16, tag="lin", name=f"ptv{blk}_{p}")
                    for h, m in ((0, 2 * p), (1, 2 * p + 1)):
                        nc.tensor.transpose(
                            pt[:, h * 128 : (h + 1) * 128],
                            vTs[m // 4][:, (m % 4) * 128 : (m % 4 + 1) * 128],
                            ident,
                        )
                    nc.vector.tensor_copy(v_augP[p][:, 0:128], pt[:, 0:128])
                    nc.vector.tensor_copy(v_augP[p][:, 130:258], pt[:, 128:256])

            def attention_core(blk, qkv, attTs, on_group_done):
                """Fronts (scores+exp+mask) and backs (e@v) pair-pipelined per
                i-group, then normalize+transpose into attTs."""
                qTs, kTs, vTs = qkv
                v_augP = v_augs[blk]
                ets = {}

                def emit_front(ig, p):
                    ps_s = score.tile([128, 1024], f32, tag="ps",
                                      name=f"ps_s{blk}_{ig}_{p}")
                    for half, m in ((0, 2 * p), (1, 2 * p + 1)):
                        nc.tensor.matmul(
                            ps_s[:, half * 512 : (half + 1) * 512],
                            kTs[m // 4][:, (m % 4) * 128 : (m % 4 + 1) * 128],
                            qTs[ig][:], start=True, stop=True,
                        )
                    et = epool.tile([128, 1024], bf16, tag="e", name=f"e{blk}_{ig}_{p}")
                    nc.scalar.activation(et[:], ps_s[:], AF.Exp, bias=negC[:])
                    mask_counter[0] += 1
                    eng = nc.gpsimd if mask_counter[0] % GP_MASK_MOD == 0 else nc.vector
                    eng.tensor_tensor(et[:], et[:], adj_t[(ig, p)][:], OP.mult)
                    ets[(ig, p)] = et

                def emit_back(ig, p, f1t):
                    et = ets.pop((ig, p))
                    for half in range(2):
                        for ic in range(4):
                            # start=True clears the whole PSUM bank: only
                            # region 0 of each f1t tile may use it
                            nc.tensor.matmul(
                                f1t[ic // 2][:, (ic % 2) * 129 : (ic % 2) * 129 + 129],
                                et[:, half * 512 + ic * 128 : half * 512 + (ic + 1) * 128],
                                v_augP[p][:, half * 130 : half * 130 + 129],
                                start=(p == 0 and half == 0 and ic % 2 == 0),
                                stop=(p == NPAIR - 1 and half == 1),
                                skip_group_check=(ic % 2 == 1),
                            )

                def normalize_group(ig, f1t):
                    for j in range(2):
                        rcp = small.tile([128, 2], f32, tag="rcp", name=f"rcp{blk}_{ig}_{j}")
                        nc.vector.reciprocal(rcp[:, 0:1], f1t[j][:, 128:129])
                        nc.vector.reciprocal(rcp[:, 1:2], f1t[j][:, 257:258])
                        tmp = small.tile([128, 256], f16, tag="attn_tmp",
                                         name=f"tmp{blk}_{ig}_{j}")
                        for h in range(2):
                            nc.vector.tensor_scalar(
                                tmp[:, h * 128 : (h + 1) * 128],
                                f1t[j][:, h * 129 : h * 129 + 128],
                                rcp[:, h : h + 1], None, OP.mult,
                            )
                        pta = linp.tile([128, 256], f16, tag="lin", name=f"pta{blk}_{ig}_{j}")
                        for h in range(2):
                            nc.tensor.transpose(
                                pta[:, h * 128 : (h + 1) * 128],
                                tmp[:, h * 128 : (h + 1) * 128], ident,
                            )
                        nc.vector.tensor_copy(
                            attTs[ig][:, j * 256 : (j + 1) * 256], pta[:]
                        )
                    on_group_done(ig)

                LEAD = 2
                seq = [(ig, p) for ig in range(NIG) for p in range(NPAIR)]
                f1ts = {}
                for idx, (ig, p) in enumerate(seq):
                    if p == 0:
                        f1ts[ig] = [
                            f1tp.tile([128, 258], f32, tag="f1t",
                                      name=f"f1t_{blk}_{ig}_{j}")
                            for j in range(2)
                        ]
                    if idx == 0:
                        for k in range(LEAD + 1):
                            emit_front(*seq[k])
                    elif idx + LEAD < len(seq):
                        emit_front(*seq[idx + LEAD])
                    emit_back(ig, p, f1ts[ig])
                    if p == NPAIR - 1:
                        normalize_group(ig, f1ts.pop(ig))

            def make_qkv(blk):
                return tuple(
                    [actp.tile([128, 512], f16, tag=f"{nm}{blk}_{c}",
                               name=f"{nm}{blk}_{c}") for c in range(4)]
                    for nm in ("qT", "kT", "vT")
                )

            hTs = [hT[:, c * 512 : (c + 1) * 512] for c in range(4)]
            att1Ts = [actp.tile([128, 512], f16, tag=f"att1T{c}", name=f"att1T_{c}") for c in range(4)]
            att2Ts = [actp.tile([128, 512], f16, tag=f"att2T{c}", name=f"att2T_{c}") for c in range(4)]
            qkv1 = make_qkv(1)
            qkv2 = make_qkv(2)

            def final_chunk(c):
                csl = slice(c * 512, (c + 1) * 512)
                ps = linp.tile([128, 512], f32, tag="lin", name=f"ps_f_{c}")
                nc.tensor.matmul(ps[:], W["wfA"], att2Ts[c][:], start=True, stop=False)
                nc.tensor.matmul(ps[:], W["wfB"], vaeT[:, csl], start=False, stop=True)
                ot = const.tile([128, 512], f16, tag=f"outT{c}", name=f"outT_{c}")
                nc.vector.tensor_scalar(ot[:], ps[:], Bv["bf"], None, OP.add)
                for h in range(2):
                    nc.sync.dma_start(outT_d[2 * c + h], ot[:, h * 256 : (h + 1) * 256])

            for c in range(4):
                qkv_chunk(1, c, hTs[c], qkv1)
            attention_core(
                1, qkv1, att1Ts,
                on_group_done=lambda ig: qkv_chunk(2, ig, att1Ts[ig][:], qkv2),
            )
            attention_core(2, qkv2, att2Ts, on_group_done=final_chunk)

    nc.finalize()
    return nc


def _host_inputs(inputs):
    """Build per-core input maps (host-side layout transforms only)."""
    h = np.asarray(inputs["h"], np.float32)
    adj = np.asarray(inputs["adj"], np.float32)
    vae = np.asarray(inputs["vae2_fetures"], np.float32)

    Wo1 = np.asarray(inputs["Wo1"], np.float64)
    bo1 = np.asarray(inputs["bo1"], np.float64)
    Wo2 = np.asarray(inputs["Wo2"], np.float64)
    bo2 = np.asarray(inputs["bo2"], np.float64)
    Wq2 = np.asarray(inputs["Wq2"], np.float64)
    Wk2 = np.asarray(inputs["Wk2"], np.float64)
    Wv2 = np.asarray(inputs["Wv2"], np.float64)
    WfA = np.asarray(inputs["Wf"], np.float64)[:, 0:128]

    Wq2f = (Wq2 @ Wo1).astype(np.float32)
    Wk2f = (Wk2 @ Wo1).astype(np.float32)
    Wv2f = (Wv2 @ Wo1).astype(np.float32)
    bq2f = (Wq2 @ bo1 + np.asarray(inputs["bq2"], np.float64)).astype(np.float32)
    bk2f = (Wk2 @ bo1 + np.asarray(inputs["bk2"], np.float64)).astype(np.float32)
    bv2f = (Wv2 @ bo1 + np.asarray(inputs["bv2"], np.float64)).astype(np.float32)
    WfAf = (WfA @ Wo2).astype(np.float32)
    bff = (WfA @ bo2 + np.asarray(inputs["bf"], np.float64)).astype(np.float32)

    wlist = [
        np.asarray(inputs["Wq1"]).T, np.asarray(inputs["Wk1"]).T,
        np.asarray(inputs["Wv1"]).T,
        Wq2f.T, Wk2f.T, Wv2f.T,
        WfAf.T, np.ascontiguousarray(np.asarray(inputs["Wf"]).T[128:256, :]),
        np.eye(128, dtype=np.float32),
    ]
    wpack = np.stack(wlist, axis=0).astype(np.float16)
    blist = [
        np.asarray(inputs["bq1"], np.float32), np.asarray(inputs["bk1"], np.float32),
        np.asarray(inputs["bv1"], np.float32),
        bq2f, bk2f, bv2f, bff,
    ]
    bpack = np.stack(blist, axis=1)

    in_maps = []
    for b in range(B):
        T = np.ascontiguousarray(adj[b].T)  # [m, i]
        # [ig, pair, 128, 1024]: pair block = [mA rows | mB rows] of ig's 512 cols
        t = T.reshape(NM, 128, NIG, 512).transpose(2, 0, 1, 3)  # [ig, m, 128, 512]
        t = t.reshape(NIG, NPAIR, 2, 128, 512).transpose(0, 1, 3, 2, 4)
        adjP = np.ascontiguousarray(t.reshape(NIG * NPAIR, 128, 1024)).astype(np_bf16)
        in_maps.append(
            {
                "hT": np.ascontiguousarray(h[b].T).astype(np.float16),
                "adjP": adjP,
                "vaeT": np.ascontiguousarray(vae[b].T).astype(np.float16),
                "wpack": wpack,
                "bpack": bpack,
            }
        )
    return in_maps


_NC_CACHE = None


def kernel(**inputs) -> np.ndarray:
    global _NC_CACHE
    if _NC_CACHE is None:
        _NC_CACHE = build_nc()
    nc = _NC_CACHE
    in_maps = _host_inputs(inputs)
    res = run_bass_kernel_spmd(nc, in_maps, list(range(NCORES)))
    outs = []
    for r in res.results:
        o = np.asarray(r["outT"], np.float32)  # [8, 128 feat, 256 node]
        outs.append(np.concatenate([o[k] for k in range(8)], axis=1).T)
    out = np.stack(outs)
    return out


# revision 17
# speedup vs baseline: 1.2540x; 1.2540x over previous
"""Trainium2 Bass kernel for the 2-block masked-attention GNN (nn_FEATURE_rec_16930761081280).

Strategy (v3)
-------------
Data-parallel over batch B=8 across 8 NeuronCores (1 graph per core).
Per core, the whole network runs out of SBUF in a transposed layout:

  - All activations are feature-major ("xT" = [128 feat, 2048 node]); every
    linear is a stationary-weight matmul chain.
  - Attention scores are computed TRANSPOSED (sT[m, i] = sum_d kT[d,m] qT[d,i])
    so softmax renormalization is deferred: the e@v contraction over m runs
    with eT chunks stationary against v_aug = [v | 1], yielding f1_unnorm and
    the row-sum in one PSUM region; normalization is a per-partition scalar
    multiply in the natural domain fused into the PSUM->SBUF move.
  - softmax uses a fixed shift C=64 (scores are relu'd-q . relu'd-k >= 0,
    bounded ~92 for this input distribution); masked entries become exact
    zeros via the multiplicative adjacency mask after exp.
  - Wo of each attention block is FOLDED into the next stage's linears on the
    host (W~q2 = Wq2 @ Wo1, b~q2 = Wq2 @ bo1 + bq2, W~fA = WfA @ Wo2,
    b~f = WfA @ bo2 + bf): the normalized+transposed attention output feeds
    the next block's q/k/v matmuls directly - no wo matmul, no wo bias.
  - Engine balance: ACT does ONLY exp (the sole exp engine at 1 col/cycle
    from fp32 PSUM - the hard floor of this kernel, ~72us).  All bias+relu,
    normalization scales and copies run on DVE; a tunable slice of the
    adjacency-mask multiplies runs on the otherwise-idle GPSIMD.  Mask
    multiplies work on merged [128,2048] double-pair tiles to halve DVE
    instruction overhead.
  - PSUM: "score" 2x[128,1024]f32, "f1t" 2x[128,258]f32 (two 129-wide
    accumulation regions per bank; matmul start=True clears the whole bank,
    so only region 0 uses start=True and region 1 relies on per-element
    has_written), "lin" 2 banks for linears/transposes.
  - ~24 warmup matmuls run during the input DMA so the PE HAM clock-gate is
    at 2.4 GHz when real work starts; block-2 q/k/v chunks and final-linear
    chunks are emitted eagerly per attention i-group (per-block tile tags
    keep the pipelines WAR-free) so the PE never idles >3.4us.

Precision: fp16 for q/k/s and linear weights (fp32 accumulate), bf16 for e/v
(exp needs the 8-bit exponent range), fp32 for biases/psum/normalization.
Measured end-to-end max-abs rel error vs the fp32 reference: ~4.8e-3.
"""

import sys

sys.path.insert(0, "/opt/trn_rl_repo")

import numpy as np
import ml_dtypes

import concourse.bass as bass
import concourse.bacc as bacc
import concourse.tile as tile
from concourse import mybir
from concourse.bass_utils import run_bass_kernel_spmd

B, N, D = 8, 2048, 128
NCORES = 8
C_SUB = 64.0  # fixed softmax shift
NM = N // 128  # 16 m-chunks
NIG = 4        # i-groups of 512
NPAIR = NM // 2
NQUAD = NPAIR // 2  # double-pairs per ig for the mask multiply

# which double-pair mask multiplies go to GPSIMD: every GP_MASK_MOD-th
GP_MASK_MOD = 10**9
N_WARM_MM = 8

f32 = mybir.dt.float32
f16 = mybir.dt.float16
bf16 = mybir.dt.bfloat16

np_bf16 = ml_dtypes.bfloat16

W_NAMES = ["wq1", "wk1", "wv1", "wq2", "wk2", "wv2", "wfA", "wfB", "ident"]
B_NAMES = ["bq1", "bk1", "bv1", "bq2", "bk2", "bv2", "bf"]


def build_nc():
    nc = bacc.Bacc(None)
    AF = mybir.ActivationFunctionType
    OP = mybir.AluOpType

    hT_d = nc.dram_tensor("hT", [D, N], f16, kind="ExternalInput")
    # adjacency double-pair tiles: [ig*NQUAD + q, 128, 2048]
    adjP_d = nc.dram_tensor("adjP", [NIG * NPAIR, 128, 1024], bf16, kind="ExternalInput")
    vaeT_d = nc.dram_tensor("vaeT", [D, N], f16, kind="ExternalInput")
    wpack_d = nc.dram_tensor("wpack", [128, len(W_NAMES) * 128], f16, kind="ExternalInput")
    bpack_d = nc.dram_tensor("bpack", [128, len(B_NAMES)], f32, kind="ExternalInput")
    outT_d = nc.dram_tensor("outT", [D, N], f16, kind="ExternalOutput")

    with tile.TileContext(nc) as tc:
        with (
            tc.tile_pool(name="const", bufs=1) as const,
            tc.tile_pool(name="adj", bufs=1) as adjp,
            tc.tile_pool(name="act", bufs=1) as actp,
            tc.tile_pool(name="small", bufs=8) as small,
            tc.tile_pool(name="e", bufs=4) as epool,
            tc.tile_pool(name="score", bufs=2, space="PSUM") as score,
            tc.tile_pool(name="f1tp", bufs=2, space="PSUM") as f1tp,
            tc.tile_pool(name="lin", bufs=2, space="PSUM") as linp,
        ):
            # ---- constants into SBUF via the sync HWDGE queues ----
            wpack = const.tile([128, len(W_NAMES) * 128], f16, tag="wpack")
            nc.sync.dma_start(wpack[:], wpack_d[:])
            hT = const.tile([D, N], f16, tag="hT")
            for c in range(4):
                nc.sync.dma_start(hT[:, c * 512 : (c + 1) * 512],
                                  hT_d[:, c * 512 : (c + 1) * 512])
            bpack = const.tile([128, len(B_NAMES)], f32, tag="bpack")
            nc.sync.dma_start(bpack[:], bpack_d[:])

            W = {
                name: wpack[:, j * 128 : (j + 1) * 128]
                for j, name in enumerate(W_NAMES)
            }
            Bv = {name: bpack[:, j : j + 1] for j, name in enumerate(B_NAMES)}
            ident = W["ident"]

            # adjacency mask tiles (double-pair), in consumption order
            adj_t = {}
            for ig in range(NIG):
                for p in range(NPAIR):
                    t = adjp.tile([128, 1024], bf16, tag=f"adj_{ig}_{p}")
                    nc.sync.dma_start(t[:], adjP_d[ig * NPAIR + p])
                    adj_t[(ig, p)] = t

            vaeT = const.tile([D, N], f16, tag="vaeT")
            nc.sync.dma_start(vaeT[:], vaeT_d[:])

            negC = const.tile([128, 1], f32, tag="negC")
            nc.gpsimd.memset(negC[:], -C_SUB)
            # warm the ACT exp table while DMAs stream (table load ~2.7us)
            actwarm = const.tile([128, 1], f32, tag="actwarm")
            nc.scalar.activation(actwarm[:], negC[:], AF.Exp)

            # ---- PE warmup for the HAM clock-gate ----
            for wi in range(N_WARM_MM):
                wps = score.tile([128, 512], f32, tag="ps", name=f"warm_{wi}")
                nc.tensor.matmul(wps[:], ident, wpack[:, 0:512], start=True, stop=True)

            # v_aug tiles, per block: [v_m | 1 | pad | v_m+1 | 1 | pad]
            v_augs = {}
            for blk in (1, 2):
                va = []
                for p in range(NPAIR):
                    t = actp.tile([128, 260], bf16, tag=f"v_augP{blk}_{p}",
                                  name=f"v_augP{blk}_{p}")
                    nc.gpsimd.memset(t[:, 128:129], 1.0)
                    nc.gpsimd.memset(t[:, 258:259], 1.0)
                    va.append(t)
                v_augs[blk] = va

            mask_counter = [0]

            def qkv_chunk(blk, c, xT_c, qkv):
                """Emit q/k/v linears for chunk c of block blk plus the v
                transposes for pairs 2c, 2c+1."""
                sfx = str(blk)
                qTs, kTs, vTs = qkv
                for w_name, b_name, dst, on_act in (
                    ("wq" + sfx, "bq" + sfx, qTs[c], True),
                    ("wk" + sfx, "bk" + sfx, kTs[c], blk == 1),
                    ("wv" + sfx, "bv" + sfx, vTs[c], False),
                ):
                    ps = linp.tile([128, 512], f32, tag="lin", name=f"ps_{w_name}_{c}")
                    nc.tensor.matmul(ps[:], W[w_name], xT_c, start=True, stop=True)
                    if on_act:
                        nc.scalar.activation(dst[:], ps[:], AF.Relu, bias=Bv[b_name])
                    else:
                        nc.vector.tensor_scalar(
                            dst[:], ps[:], Bv[b_name], 0.0, OP.add, OP.max
                        )
                v_augP = v_augs[blk]
                for p in (2 * c, 2 * c + 1):
                    pt = linp.tile([128, 256], f16, tag="lin", name=f"ptv{blk}_{p}")
                    for h, m in ((0, 2 * p), (1, 2 * p + 1)):
                        nc.tensor.transpose(
                            pt[:, h * 128 : (h + 1) * 128],
                            vTs[m // 4][:, (m % 4) * 128 : (m % 4 + 1) * 128],
                            ident,
                        )
                    nc.vector.tensor_copy(v_augP[p][:, 0:128], pt[:, 0:128])
                    nc.vector.tensor_copy(v_augP[p][:, 130:258], pt[:, 128:256])

            def attention_core(blk, qkv, attTs, on_group_done):
                """Fronts (scores+exp+mask) and backs (e@v) pair-pipelined per
                i-group, then normalize+transpose into attTs."""
                qTs, kTs, vTs = qkv
                v_augP = v_augs[blk]
                ets = {}

                def emit_front(ig, p):
                    ps_s = score.tile([128, 1024], f32, tag="ps",
                                      name=f"ps_s{blk}_{ig}_{p}")
                    for half, m in ((0, 2 * p), (1, 2 * p + 1)):
                        nc.tensor.matmul(
                            ps_s[:, half * 512 : (half + 1) * 512],
                            kTs[m // 4][:, (m % 4) * 128 : (m % 4 + 1) * 128],
                            qTs[ig][:], start=True, stop=True,
                        )
                    et = epool.tile([128, 1024], bf16, tag="e", name=f"e{blk}_{ig}_{p}")
                    nc.scalar.activation(et[:], ps_s[:], AF.Exp, bias=negC[:])
                    mask_counter[0] += 1
                    eng = nc.gpsimd if mask_counter[0] % GP_MASK_MOD == 0 else nc.vector
                    eng.tensor_tensor(et[:], et[:], adj_t[(ig, p)][:], OP.mult)
                    ets[(ig, p)] = et

                def emit_back(ig, p, f1t):
                    et = ets.pop((ig, p))
                    for half in range(2):
                        for ic in range(4):
                            # start=True clears the whole PSUM bank: only
                            # region 0 of each f1t tile may use it
                            nc.tensor.matmul(
                                f1t[ic // 2][:, (ic % 2) * 129 : (ic % 2) * 129 + 129],
                                et[:, half * 512 + ic * 128 : half * 512 + (ic + 1) * 128],
                                v_augP[p][:, half * 130 : half * 130 + 129],
                                start=(p == 0 and half == 0 and ic % 2 == 0),
                                stop=(p == NPAIR - 1 and half == 1),
                                skip_group_check=(ic % 2 == 1),
                            )

                def normalize_group(ig, f1t):
                    for j in range(2):
                        rcp = small.tile([128, 2], f32, tag="rcp", name=f"rcp{blk}_{ig}_{j}")
                        nc.vector.reciprocal(rcp[:, 0:1], f1t[j][:, 128:129])
                        nc.vector.reciprocal(rcp[:, 1:2], f1t[j][:, 257:258])
                        tmp = small.tile([128, 256], f16, tag="attn_tmp",
                                         name=f"tmp{blk}_{ig}_{j}")
                        for h in range(2):
                            nc.vector.tensor_scalar(
                                tmp[:, h * 128 : (h + 1) * 128],
                                f1t[j][:, h * 129 : h * 129 + 128],
                                rcp[:, h : h + 1], None, OP.mult,
                            )
                        pta = linp.tile([128, 256], f16, tag="lin", name=f"pta{blk}_{ig}_{j}")
                        for h in range(2):
                            nc.tensor.transpose(
                                pta[:, h * 128 : (h + 1) * 128],
                                tmp[:, h * 128 : (h + 1) * 128], ident,
                            )
                        nc.vector.tensor_copy(
                            attTs[ig][:, j * 256 : (j + 1) * 256], pta[:]
                        )
                    on_group_done(ig)

                LEAD = 2
                seq = [(ig, p) for ig in range(NIG) for p in range(NPAIR)]
                f1ts = {}
                for idx, (ig, p) in enumerate(seq):
                    if p == 0:
                        f1ts[ig] = [
                            f1tp.tile([128, 258], f32, tag="f1t",
                                      name=f"f1t_{blk}_{ig}_{j}")
                            for j in range(2)
                        ]
                    if idx == 0:
                        for k in range(LEAD + 1):
                            emit_front(*seq[k])
                    elif idx + LEAD < len(seq):
                        emit_front(*seq[idx + LEAD])
                    emit_back(ig, p, f1ts[ig])
                    if p == NPAIR - 1:
                        normalize_group(ig, f1ts.pop(ig))

            def make_qkv(blk):
                return tuple(
                    [actp.tile([128, 512], f16, tag=f"{nm}{blk}_{c}",
                               name=f"{nm}{blk}_{c}") for c in range(4)]
                    for nm in ("qT", "kT", "vT")
                )

            hTs = [hT[:, c * 512 : (c + 1) * 512] for c in range(4)]
            att1Ts = [actp.tile([128, 512], f16, tag=f"att1T{c}", name=f"att1T_{c}") for c in range(4)]
            att2Ts = [actp.tile([128, 512], f16, tag=f"att2T{c}", name=f"att2T_{c}") for c in range(4)]
            qkv1 = make_qkv(1)
            qkv2 = make_qkv(2)

            def final_chunk(c):
                csl = slice(c * 512, (c + 1) * 512)
                ps = linp.tile([128, 512], f32, tag="lin", name=f"ps_f_{c}")
                nc.tensor.matmul(ps[:], W["wfA"], att2Ts[c][:], start=True, stop=False)
                nc.tensor.matmul(ps[:], W["wfB"], vaeT[:, csl], start=False, stop=True)
                ot = const.tile([128, 512], f16, tag=f"outT{c}", name=f"outT_{c}")
                nc.vector.tensor_scalar(ot[:], ps[:], Bv["bf"], None, OP.add)
                nc.sync.dma_start(outT_d[:, csl], ot[:])

            for c in range(4):
                qkv_chunk(1, c, hTs[c], qkv1)
            attention_core(
                1, qkv1, att1Ts,
                on_group_done=lambda ig: qkv_chunk(2, ig, att1Ts[ig][:], qkv2),
            )
            attention_core(2, qkv2, att2Ts, on_group_done=final_chunk)

    nc.finalize()
    return nc


def _host_inputs(inputs):
    """Build per-core input maps (host-side layout transforms only)."""
    h = np.asarray(inputs["h"], np.float32)
    adj = np.asarray(inputs["adj"], np.float32)
    vae = np.asarray(inputs["vae2_fetures"], np.float32)

    Wo1 = np.asarray(inputs["Wo1"], np.float64)
    bo1 = np.asarray(inputs["bo1"], np.float64)
    Wo2 = np.asarray(inputs["Wo2"], np.float64)
    bo2 = np.asarray(inputs["bo2"], np.float64)
    Wq2 = np.asarray(inputs["Wq2"], np.float64)
    Wk2 = np.asarray(inputs["Wk2"], np.float64)
    Wv2 = np.asarray(inputs["Wv2"], np.float64)
    WfA = np.asarray(inputs["Wf"], np.float64)[:, 0:128]

    Wq2f = (Wq2 @ Wo1).astype(np.float32)
    Wk2f = (Wk2 @ Wo1).astype(np.float32)
    Wv2f = (Wv2 @ Wo1).astype(np.float32)
    bq2f = (Wq2 @ bo1 + np.asarray(inputs["bq2"], np.float64)).astype(np.float32)
    bk2f = (Wk2 @ bo1 + np.asarray(inputs["bk2"], np.float64)).astype(np.float32)
    bv2f = (Wv2 @ bo1 + np.asarray(inputs["bv2"], np.float64)).astype(np.float32)
    WfAf = (WfA @ Wo2).astype(np.float32)
    bff = (WfA @ bo2 + np.asarray(inputs["bf"], np.float64)).astype(np.float32)

    wlist = [
        np.asarray(inputs["Wq1"]).T, np.asarray(inputs["Wk1"]).T,
        np.asarray(inputs["Wv1"]).T,
        Wq2f.T, Wk2f.T, Wv2f.T,
        WfAf.T, np.ascontiguousarray(np.asarray(inputs["Wf"]).T[128:256, :]),
        np.eye(128, dtype=np.float32),
    ]
    wpack = np.concatenate(wlist, axis=1).astype(np.float16)
    blist = [
        np.asarray(inputs["bq1"], np.float32), np.asarray(inputs["bk1"], np.float32),
        np.asarray(inputs["bv1"], np.float32),
        bq2f, bk2f, bv2f, bff,
    ]
    bpack = np.stack(blist, axis=1)

    in_maps = []
    for b in range(B):
        T = np.ascontiguousarray(adj[b].T)  # [m, i]
        # [ig, pair, 128, 1024]: pair block = [mA rows | mB rows] of ig's 512 cols
        t = T.reshape(NM, 128, NIG, 512).transpose(2, 0, 1, 3)  # [ig, m, 128, 512]
        t = t.reshape(NIG, NPAIR, 2, 128, 512).transpose(0, 1, 3, 2, 4)
        adjP = np.ascontiguousarray(t.reshape(NIG * NPAIR, 128, 1024)).astype(np_bf16)
        in_maps.append(
            {
                "hT": np.ascontiguousarray(h[b].T).astype(np.float16),
                "adjP": adjP,
                "vaeT": np.ascontiguousarray(vae[b].T).astype(np.float16),
                "wpack": wpack,
                "bpack": bpack,
            }
        )
    return in_maps


_NC_CACHE = None


def kernel(**inputs) -> np.ndarray:
    global _NC_CACHE
    if _NC_CACHE is None:
        _NC_CACHE = build_nc()
    nc = _NC_CACHE
    in_maps = _host_inputs(inputs)
    res = run_bass_kernel_spmd(nc, in_maps, list(range(NCORES)))
    out = np.stack([np.asarray(r["outT"], np.float32).T for r in res.results])
    return out
